# revision 1
# baseline (speedup 1.0000x reference)
"""2-layer GCN (PyG GCNConv semantics) on 8 TRN2 NeuronCores — dma_gather rewrite.

Strategy:
- Nodes sharded contiguously across 8 cores (12500 rows each), degree-sorted
  within each core so per-tile edge counts align across cores; permutation
  undone on the host.
- Layer math: h~ = D^-1/2 (x @ W1); table = AllGather(h~) in HBM (bf16,
  row-contiguous); agg[d] = D^-1/2[d] * sum_{e: dst=d} table[src_e]
  (self-loops are edges). Layer 2 repeats on AllGather(D^-1/2 relu(agg+b1)),
  then out = log_softmax(agg2 @ W2 + b2).
- The irregular gather uses GPSIMD dma_gather: thousands of 256B rows per
  instruction (vs one [128,1] indirect DMA per slot column before). int16
  indices reach 32768 strides, so the table is addressed in strides of 4
  nodes (512B); each edge gathers 256B at byte offset (src%4)*128 — the
  target node's 128B row is always the first half of the slot. Streams are
  split per (dst-tile, src%4 residue) so each gather instruction has a
  single base view.
- Aggregation is exact (no per-destination padded widths): gathered slots
  land 128-per-block on partitions; a per-tile indicator [e, d] =
  (dstlocal[e] == d) is built in one DVE is_equal with broadcast APs, and
  per-block matmuls (lhsT=indicator, rhs=slot first halves) accumulate the
  per-tile segment sum in PSUM. Pad slots point at stride 0 with
  dstlocal=200 -> indicator column 0 -> no contribution.
- log_softmax ln() via Newton iteration on ScalarE Exp (no Ln ACT table).
"""
import sys

sys.path.insert(0, "/opt/trn_rl_repo")

import numpy as np

import concourse.bass as bass
import concourse.bacc as bacc
import concourse.tile as tile
import concourse.mybir as mybir
from concourse import bass_utils
from concourse.masks import make_identity
from concourse.tile import add_dep_helper

N = 100000
F = 512
H = 64
CLS = 10
NC = 8
NPC = N // NC          # 12500 nodes per core
P = 128
NT = (NPC + P - 1) // P  # 98 tiles per core
NPAD = NT * P            # 12544
RG = [list(range(NC))]
NRES = 4                 # src % 4 residues (int16 idx = src // 4 < 25000)
SMAX = (N + NRES - 1) // NRES  # 25000 strides
TBL_ELEMS = N * H + 256        # flat bf16 table + tail pad for r=3 over-read
GB = 32                  # gather chunk size in 128-slot blocks
TG = 14                  # tiles per phase-A transpose group (98 = 7*14)

BF16 = mybir.dt.bfloat16
F32 = mybir.dt.float32
I16 = mybir.dt.int16
FP8 = mybir.dt.float8e4
NP_BF16 = mybir.dt.np(BF16)
NP_FP8 = mybir.dt.np(FP8)

_cache = {}


def _plan_chunks(L):
    """Group consecutive tiles per residue into gather chunks of <= GB blocks.

    L: [NT][NRES] slot counts (multiples of 128).
    Returns per residue: list of chunks [(idx_col0, nidx, {t: block_off})],
    plus per (t, r): (chunk_id, block_off) and ring idx-stream column offsets.
    """
    chunks = []
    where = {}
    col0 = 0
    for r in range(NRES):
        ring = []
        cur = {}
        cur_blocks = 0
        cur_col0 = col0
        for t in range(NT):
            b = L[t][r] // P
            if b == 0:
                continue
            if cur_blocks + b > GB and cur_blocks > 0:
                ring.append((cur_col0, cur_blocks * P, cur))
                cur_col0 += cur_blocks * P // 16
                cur = {}
                cur_blocks = 0
            cur[t] = cur_blocks
            where[(t, r)] = (len(ring), cur_blocks)
            cur_blocks += b
        if cur_blocks > 0:
            ring.append((cur_col0, cur_blocks * P, cur))
            cur_col0 += cur_blocks * P // 16
        col0 = cur_col0
        chunks.append(ring)
    idx_cols = col0
    return chunks, where, idx_cols


def _build(Lkey):
    """Build + compile the SPMD graph for slot schedule L[t][r]."""
    L = [list(row) for row in Lkey]
    Bt = [sum(row) // P for row in L]      # blocks per tile
    B_tot = sum(Bt)
    BMAX = max(Bt)
    chunks, where, idx_cols = _plan_chunks(L)

    nc = bacc.Bacc("TRN2", target_bir_lowering=False, debug=False, num_devices=NC,
                   num_swdge_queues=4)

    x_in = nc.dram_tensor("x_in", [NPAD, F], BF16, kind="ExternalInput")
    w1_in = nc.dram_tensor("w1_in", [F, H], BF16, kind="ExternalInput")
    b1_in = nc.dram_tensor("b1_in", [P, H], F32, kind="ExternalInput")
    w2_in = nc.dram_tensor("w2_in", [H, CLS], BF16, kind="ExternalInput")
    b2_in = nc.dram_tensor("b2_in", [P, CLS], F32, kind="ExternalInput")
    dinv_in = nc.dram_tensor("dinv_in", [P, NT], F32, kind="ExternalInput")
    idx_in = nc.dram_tensor("idx_in", [P, idx_cols], I16, kind="ExternalInput")
    ind_in = nc.dram_tensor("ind_in", [P, B_tot * P], FP8, kind="ExternalInput")
    out = nc.dram_tensor("out", [NPAD, CLS], F32, kind="ExternalOutput")

    bounce1 = nc.dram_tensor("bounce1", [NPC * H], BF16, kind="Internal")
    bounce2 = nc.dram_tensor("bounce2", [NPC * H], BF16, kind="Internal")
    table1 = nc.dram_tensor("table1", [TBL_ELEMS], BF16, kind="Internal",
                            addr_space="Shared")
    table2 = nc.dram_tensor("table2", [TBL_ELEMS], BF16, kind="Internal",
                            addr_space="Shared")

    AF = mybir.ActivationFunctionType
    ALU = mybir.AluOpType

    def res_view(tbl, r):
        # rows of 128 elems (256B) at elem offset r*64, stride 256 elems (512B)
        return tbl.ap()[r * H: r * H + SMAX * 4 * H].rearrange(
            "(s c) -> s c", c=4 * H)[:, 0:2 * H]

    with tile.TileContext(nc) as tc:
        with (
            tc.tile_pool(name="const", bufs=1) as constp,
            tc.tile_pool(name="xt", bufs=2) as xtp,
            tc.tile_pool(name="hpsum", bufs=3, space="PSUM") as hpsum,
            tc.tile_pool(name="hsb", bufs=4) as hsb,
            tc.tile_pool(name="idxp", bufs=5) as idxp,
            tc.tile_pool(name="ring0", bufs=4) as ring0,
            tc.tile_pool(name="ring1", bufs=4) as ring1,
            tc.tile_pool(name="ring2", bufs=4) as ring2,
            tc.tile_pool(name="ring3", bufs=4) as ring3,
            tc.tile_pool(name="indp", bufs=4) as indp,
            tc.tile_pool(name="smp", bufs=4) as smp,
            tc.tile_pool(name="psum2", bufs=2, space="PSUM") as psum2,
        ):
            rings = [ring0, ring1, ring2, ring3]
            # --- constants ---
            w1s = constp.tile([P, 4 * H], BF16)
            for k in range(4):
                nc.sync.dma_start(w1s[:, k * H:(k + 1) * H],
                                  w1_in.ap()[k * P:(k + 1) * P, :])
            b1s = constp.tile([P, H], F32)
            nc.sync.dma_start(b1s[:], b1_in.ap())
            w2s = constp.tile([H, CLS], BF16)
            nc.sync.dma_start(w2s[:], w2_in.ap())
            b2s = constp.tile([P, CLS], F32)
            nc.sync.dma_start(b2s[:], b2_in.ap())
            dinvs = constp.tile([P, NT], F32)
            nc.sync.dma_start(dinvs[:], dinv_in.ap())
            ident = constp.tile([P, P], BF16)
            make_identity(nc, ident[:])

            # --- phase A: h1~ = dinv * (x @ W1) -> bounce1 ---
            b1v = bounce1.ap()[:].rearrange("(n f) -> n f", f=H)
            for g in range(NT // TG):
                xts = []
                for k in range(4):
                    xt = xtp.tile([P, TG * P], BF16, tag=f"xt{k}")
                    nc.sync.dma_start_transpose(
                        xt[:], x_in.ap()[g * TG * P:(g + 1) * TG * P,
                                         k * P:(k + 1) * P])
                    xts.append(xt)
                for j in range(TG):
                    t = g * TG + j
                    rows = min(P, NPC - t * P)
                    ps = hpsum.tile([P, H], F32, tag="hps")
                    for k in range(4):
                        nc.tensor.matmul(ps[:], lhsT=xts[k][:, j * P:(j + 1) * P],
                                         rhs=w1s[:, k * H:(k + 1) * H],
                                         start=(k == 0), stop=(k == 3))
                    h1 = hsb.tile([P, H], BF16, tag="h1")
                    nc.vector.tensor_scalar(out=h1[:], in0=ps[:],
                                            scalar1=dinvs[:, t:t + 1],
                                            scalar2=None, op0=ALU.mult)
                    nc.sync.dma_start(b1v[t * P:t * P + rows, :], h1[:rows, :])

            nc.gpsimd.collective_compute(
                "AllGather", ALU.bypass, replica_groups=RG,
                ins=[bounce1.ap()[:].opt()],
                outs=[table1.ap()[0:N * H].opt()])

            # --- aggregation layers ---
            def agg_layer(table, post):
                views = [res_view(table, r) for r in range(NRES)]
                chunk_tiles = [dict() for _ in range(NRES)]
                chunk_insts = [dict() for _ in range(NRES)]

                def issue_chunk(r, ci):
                    col0, nidx, _tmap = chunks[r][ci]
                    cols = nidx // 16
                    it = idxp.tile([P, GB * 8], I16, tag=f"idx{r}")
                    nc.sync.dma_start(it[:, 0:cols], idx_in.ap()[:, col0:col0 + cols])
                    mt = rings[r].tile([P, GB * P], BF16, tag=f"msg{r}")
                    gi = nc.gpsimd.dma_gather(
                        mt[:, 0:nidx].rearrange("p (b f) -> p b f", f=P),
                        views[r], it[:, 0:cols], nidx, nidx, P,
                        elem_step=4 * H, single_packet=False, queue_num=r)
                    chunk_tiles[r][ci] = mt
                    chunk_insts[r][ci] = gi

                for r in range(NRES):
                    issue_chunk(r, 0)
                issued = [1] * NRES

                boff = 0
                for t in range(NT):
                    rows = min(P, NPC - t * P)
                    # prefetch next chunks needed soon
                    for r in range(NRES):
                        if (t, r) in where:
                            ci, _ = where[(t, r)]
                            while issued[r] <= ci + 3 and issued[r] < len(chunks[r]):
                                issue_chunk(r, issued[r])
                                issued[r] += 1
                    bt = Bt[t]
                    ind = indp.tile([P, BMAX * P], FP8, tag="ind")
                    nc.scalar.dma_start(ind[:, 0:bt * P],
                                        ind_in.ap()[:, boff * P:(boff + bt) * P])
                    ps = hpsum.tile([P, H], F32, tag="hps")
                    bi = 0
                    nb = sum(L[t][r] // P for r in range(NRES))
                    for r in range(NRES):
                        nbr = L[t][r] // P
                        if nbr == 0:
                            continue
                        ci, bo = where[(t, r)]
                        mt = chunk_tiles[r][ci]
                        gi = chunk_insts[r][ci]
                        first_mm = None
                        for b in range(nbr):
                            mm = nc.tensor.matmul(
                                ps[:],
                                lhsT=ind[:, (bi + b) * P:(bi + b + 1) * P],
                                rhs=mt[:, (bo + b) * P:(bo + b) * P + H],
                                start=(bi + b == 0), stop=(bi + b == nb - 1))
                            if first_mm is None:
                                first_mm = mm
                                add_dep_helper(mm.ins, gi.ins,
                                               reason="matmul waits gather")
                        bi += nbr
                    post(t, rows, ps)
                    boff += bt

            # layer-1 epilogue: h2 = dinv * relu(dinv*agg + b1) -> bounce2
            b2v = bounce2.ap()[:].rearrange("(n f) -> n f", f=H)

            def post1(t, rows, ps):
                y = smp.tile([P, H], F32, tag="y")
                nc.vector.tensor_scalar(out=y[:], in0=ps[:],
                                        scalar1=dinvs[:, t:t + 1], scalar2=None,
                                        op0=ALU.mult)
                y2 = smp.tile([P, H], F32, tag="y2")
                nc.vector.tensor_tensor(out=y2[:], in0=y[:], in1=b1s[:], op=ALU.add)
                h2 = hsb.tile([P, H], BF16, tag="h2")
                nc.vector.tensor_scalar(out=h2[:], in0=y2[:], scalar1=0.0,
                                        scalar2=dinvs[:, t:t + 1],
                                        op0=ALU.max, op1=ALU.mult)
                nc.sync.dma_start(b2v[t * P:t * P + rows, :], h2[:rows, :])

            agg_layer(table1, post1)

            nc.gpsimd.collective_compute(
                "AllGather", ALU.bypass, replica_groups=RG,
                ins=[bounce2.ap()[:].opt()],
                outs=[table2.ap()[0:N * H].opt()])

            # layer-2 epilogue: out = log_softmax(dinv*agg @ W2 + b2)
            # yo accumulated per SG-tile group, softmax chain once per group
            SG = 7
            grp = {"buf": None, "t0": 0, "n": 0}

            def flush_group():
                ng = grp["n"]
                if ng == 0:
                    return
                yb = grp["buf"]
                t0 = grp["t0"]
                mx = smp.tile([P, SG], F32, tag="mx")
                nc.vector.tensor_reduce(
                    out=mx[:, 0:ng],
                    in_=yb[:, 0:ng * CLS].rearrange("p (j c) -> p j c", c=CLS),
                    axis=mybir.AxisListType.X, op=ALU.max)
                sh = smp.tile([P, SG * CLS], F32, tag="sh")
                nc.vector.tensor_tensor(
                    out=sh[:, 0:ng * CLS].rearrange("p (j c) -> p j c", c=CLS),
                    in0=yb[:, 0:ng * CLS].rearrange("p (j c) -> p j c", c=CLS),
                    in1=mx[:, 0:ng].unsqueeze(2).broadcast_to([P, ng, CLS]),
                    op=ALU.subtract)
                ex = smp.tile([P, SG * CLS], F32, tag="ex")
                nc.scalar.activation(out=ex[:, 0:ng * CLS], in_=sh[:, 0:ng * CLS],
                                     func=AF.Exp)
                sm = smp.tile([P, SG], F32, tag="sm")
                nc.vector.tensor_reduce(
                    out=sm[:, 0:ng],
                    in_=ex[:, 0:ng * CLS].rearrange("p (j c) -> p j c", c=CLS),
                    axis=mybir.AxisListType.X, op=ALU.add)
                # ls = ln(sm): Newton on f(y) = e^y - sm
                ls = smp.tile([P, SG], F32, tag="ls")
                nc.vector.tensor_scalar(out=ls[:, 0:ng], in0=sm[:, 0:ng],
                                        scalar1=0.2559, scalar2=-0.2559,
                                        op0=ALU.mult, op1=ALU.add)
                for _ in range(3):
                    en = smp.tile([P, SG], F32, tag="en")
                    nc.scalar.activation(out=en[:, 0:ng], in_=ls[:, 0:ng],
                                         func=AF.Exp, scale=-1.0)
                    pr = smp.tile([P, SG], F32, tag="pr")
                    nc.vector.tensor_tensor(out=pr[:, 0:ng], in0=en[:, 0:ng],
                                            in1=sm[:, 0:ng], op=ALU.mult)
                    ls2 = smp.tile([P, SG], F32, tag="ls")
                    nc.vector.tensor_tensor(out=ls2[:, 0:ng], in0=ls[:, 0:ng],
                                            in1=pr[:, 0:ng], op=ALU.add)
                    ls = ls2
                    nc.vector.tensor_scalar(out=ls[:, 0:ng], in0=ls[:, 0:ng],
                                            scalar1=1.0, scalar2=None,
                                            op0=ALU.subtract)
                res = smp.tile([P, SG * CLS], F32, tag="res")
                nc.vector.tensor_tensor(
                    out=res[:, 0:ng * CLS].rearrange("p (j c) -> p j c", c=CLS),
                    in0=sh[:, 0:ng * CLS].rearrange("p (j c) -> p j c", c=CLS),
                    in1=ls[:, 0:ng].unsqueeze(2).broadcast_to([P, ng, CLS]),
                    op=ALU.subtract)
                for j in range(ng):
                    t = t0 + j
                    rows = min(P, NPC - t * P)
                    nc.sync.dma_start(out.ap()[t * P:t * P + rows, :],
                                      res[:rows, j * CLS:(j + 1) * CLS])
                grp["buf"] = None
                grp["n"] = 0

            def post2(t, rows, ps):
                aggb = smp.tile([P, H], BF16, tag="aggb")
                nc.vector.tensor_scalar(out=aggb[:], in0=ps[:],
                                        scalar1=dinvs[:, t:t + 1], scalar2=None,
                                        op0=ALU.mult)
                pt = psum2.tile([H, P], BF16, tag="pt")
                nc.tensor.transpose(out=pt[:], in_=aggb[:], identity=ident[:])
                aggT = smp.tile([H, P], BF16, tag="aggT")
                nc.vector.tensor_copy(out=aggT[:], in_=pt[:])
                po = psum2.tile([P, CLS], F32, tag="po")
                nc.tensor.matmul(po[:], lhsT=aggT[:], rhs=w2s[:], start=True,
                                 stop=True)
                if grp["buf"] is None:
                    grp["buf"] = smp.tile([P, SG * CLS], F32, tag="yb", name="yb")
                    grp["t0"] = t
                j = grp["n"]
                nc.vector.tensor_tensor(
                    out=grp["buf"][:, j * CLS:(j + 1) * CLS], in0=po[:],
                    in1=b2s[:], op=ALU.add)
                grp["n"] += 1
                if grp["n"] == SG:
                    flush_group()

            agg_layer(table2, post2)
            flush_group()

    nc.compile()
    return nc


def _prep(x, edge_index, W1, b1, W2, b2):
    """Host-side graph preprocessing."""
    x = np.asarray(x, dtype=np.float32)
    ei = np.asarray(edge_index, dtype=np.int64)
    W1 = np.asarray(W1, dtype=np.float32)
    b1 = np.asarray(b1, dtype=np.float32)
    W2 = np.asarray(W2, dtype=np.float32)
    b2 = np.asarray(b2, dtype=np.float32)

    nodes = np.arange(N, dtype=np.int64)
    src_f = np.concatenate([nodes, ei[0]])
    dst_f = np.concatenate([nodes, ei[1]])
    deg = np.bincount(dst_f, minlength=N)  # >= 1 (self-loops)
    dinv = (1.0 / np.sqrt(deg)).astype(np.float32)

    # per-core degree-descending permutation (tiles align across cores)
    order = np.argsort(-deg.reshape(NC, NPC), axis=1, kind="stable")
    perm_global = (np.arange(NC, dtype=np.int64)[:, None] * NPC + order)
    pos_of = np.empty(N, np.int64)
    pos_of[perm_global.ravel()] = np.arange(N, dtype=np.int64)

    spos = pos_of[src_f]
    dpos = pos_of[dst_f]
    c = dpos // NPC
    loc = dpos % NPC
    t_arr = loc // P
    p_arr = loc % P
    r_arr = spos % NRES
    q_arr = spos // NRES  # int16 idx value

    key = ((c * NT + t_arr) * NRES + r_arr)
    sidx = np.argsort(key, kind="stable")
    key_s = key[sidx]
    q_s = q_arr[sidx]
    p_s = p_arr[sidx]

    cnt = np.bincount(key_s, minlength=NC * NT * NRES).reshape(NC, NT, NRES)
    M = cnt.max(axis=0)                                   # [NT, NRES]
    L = ((M + P - 1) // P * P).astype(np.int64)           # [NT, NRES]
    Bt = (L.sum(axis=1) // P)                             # [NT]
    B_tot = int(Bt.sum())

    # ring stream offsets (slots) per (t, r) within ring r
    ring_off = np.zeros((NT, NRES), np.int64)
    for r in range(NRES):
        ring_off[1:, r] = np.cumsum(L[:-1, r])
    SL = L.sum(axis=0)            # slots per ring
    SL_tot = int(SL.sum())
    ring_base = np.zeros(NRES + 1, np.int64)
    np.cumsum(SL, out=ring_base[1:])

    # per-edge position within its (c, t, r) group
    starts = np.zeros(NC * NT * NRES + 1, np.int64)
    np.cumsum(cnt.ravel(), out=starts[1:])
    j = np.arange(len(key_s), dtype=np.int64) - starts[key_s]
    t_e = (key_s // NRES) % NT
    r_e = key_s % NRES
    c_e = key_s // (NT * NRES)
    gpos = ring_base[r_e] + ring_off[t_e, r_e] + j        # ring-stream slot

    idx_all = np.zeros((NC, SL_tot), np.int16)
    idx_all[c_e, gpos] = q_s.astype(np.int16)

    # dstlocal tile-major grid [P, B_tot]
    tile_base = np.zeros(NT, np.int64)
    tile_base[1:] = np.cumsum(Bt[:-1] * P)
    res_off = np.zeros((NT, NRES), np.int64)
    for r in range(1, NRES):
        res_off[:, r] = res_off[:, r - 1] + L[:, r - 1]
    tpos = tile_base[t_e] + res_off[t_e, r_e] + j
    ind_all = np.zeros((NC, P, B_tot * P), np.uint8)
    ind_all[c_e, tpos % P, (tpos // P) * P + p_s] = 0x38  # 1.0 in fp8 e4m3

    # wrap idx streams: position g -> [g%16, g//16], replicated 8x
    idx_wrap = idx_all.reshape(NC, SL_tot // 16, 16).transpose(0, 2, 1)
    idx_wrap = np.tile(idx_wrap, (1, 8, 1))               # [NC, 128, SL/16]

    dinv_perm = dinv[perm_global]
    dinv_pad = np.ones((NC, NPAD), np.float32)
    dinv_pad[:, :NPC] = dinv_perm
    dinvT = np.ascontiguousarray(
        dinv_pad.reshape(NC, NT, P).transpose(0, 2, 1))   # [NC, P, NT]

    W1_bf = np.ascontiguousarray(W1.astype(NP_BF16))
    W2_bf = np.ascontiguousarray(W2.astype(NP_BF16))
    b1_bc = np.ascontiguousarray(
        np.broadcast_to(b1[None, :], (P, H)).astype(np.float32))
    b2_bc = np.ascontiguousarray(
        np.broadcast_to(b2[None, :], (P, CLS)).astype(np.float32))

    in_maps = []
    for cc in range(NC):
        xp = np.zeros((NPAD, F), dtype=NP_BF16)
        xp[:NPC] = x[perm_global[cc]].astype(NP_BF16)
        in_maps.append({
            "x_in": xp,
            "w1_in": W1_bf,
            "b1_in": b1_bc,
            "w2_in": W2_bf,
            "b2_in": b2_bc,
            "dinv_in": np.ascontiguousarray(dinvT[cc]),
            "idx_in": np.ascontiguousarray(idx_wrap[cc]),
            "ind_in": ind_all[cc].view(NP_FP8),
        })
    Lkey = tuple(tuple(int(v) for v in row) for row in L)
    return Lkey, in_maps, order


def _get_nc(Lkey):
    if Lkey not in _cache:
        _cache[Lkey] = _build(Lkey)
    return _cache[Lkey]


def run(x, edge_index, W1, b1, W2, b2, trace=False):
    Lkey, in_maps, order = _prep(x, edge_index, W1, b1, W2, b2)
    nc = _get_nc(Lkey)
    res = bass_utils.run_bass_kernel_spmd(
        nc, in_maps, core_ids=list(range(NC)), trace=trace)
    out_full = np.empty((N, CLS), np.float32)
    for c in range(NC):
        oc = res.results[c]["out"][:NPC]
        out_full[c * NPC + order[c]] = oc
    return out_full, res


def kernel(x, edge_index, W1, b1, W2, b2):
    out_full, _ = run(x, edge_index, W1, b1, W2, b2)
    return out_full

